# revision 36
# baseline (speedup 1.0000x reference)
"""Trainium2 Bass kernel for the BPR-style soft-label pairwise loss.

Reference math (per graph g of B=16, N=2048 nodes, labels in {0..3}):
  for lvl in 1..3:
    s_lvl   = sum_{i: lab=lvl} sum_{j: lab<lvl} log_sigmoid(x_i - x_j)
    cnt_lvl = n_lvl * n_{<lvl};  mean_lvl = s_lvl/cnt_lvl if cnt>0 else 0
  per_graph = sum(mean_lvl) / max(#valid, 1);  loss = -mean_g(per_graph)

Kernel strategy (data-parallel, 2 graphs per core on 8 cores):
  -log_sigmoid(x_i - x_j) = ln(1 + e^{x_j} * e^{-x_i})
  The host sorts each graph's nodes by label into a class-segmented layout
  that is uniform across graphs (segment size = max class count over all
  graphs rounded to even; padded slots carry e^{x}=0 so ln(1+0)=0 contributes
  nothing).  Only pairs with lab_i > lab_j are ever evaluated: i-tiles cover
  classes 1..3, each with j-extent = end of class (lab_i - 1)'s segment, so
  the device does ~3/8 of the dense N^2 transcendental work.

  Per 128-row i-tile the DVE forms t = xrep * e^{-x_i} (bf16 tensor_scalar,
  4x mode) into a grouped buffer; one ScalarE Ln instruction (bias=1) covers
  a whole group of tiles (ACT cost is per-column, so fewer instructions means
  less fixed overhead).  A one-hot [128,4] bf16 matmul contracts the i
  dimension class-resolved into PSUM, with each 512-wide j-chunk accumulating
  into its own 4-row partition band of a single PSUM bank; one DVE copy
  drains a level's whole G to SBUF, a DMA exports it, and the O(N) segment
  sums plus O(1) count/divide/average logic run on host in float64.
"""

import os
import sys

import numpy as np

for _p in ("/root/.axon_site/_ro/trn_rl_repo", "/opt/trn_rl_repo"):
    if os.path.isdir(_p) and _p not in sys.path:
        sys.path.append(_p)

import concourse.bacc as bacc
import concourse.mybir as mybir
import concourse.tile as tile
from concourse.bass_utils import run_bass_kernel_spmd

B, N, NCLS = 16, 2048, 4
N_CORES = 8
GPC = B // N_CORES  # graphs per core
P = 128
CH = 512           # PSUM bank chunk (f32 columns)
AF = mybir.ActivationFunctionType

_BUILD_CACHE = {}


def _layout(scls):
    """Derive the uniform class-segmented layout from per-class segment sizes."""
    s0, s1, s2, s3 = scls
    jstart = [0, s0, s0 + s1, s0 + s1 + s2]  # segment starts for j classes 0..2
    lj = s0 + s1 + s2                        # j layout length (classes 0..2)
    jext = {1: jstart[1], 2: jstart[2], 3: lj}  # j extent per i level
    istart = {1: 0, 2: s1, 3: s1 + s2}       # i layout: classes 1..3
    li_raw = s1 + s2 + s3
    ti = max(0, -(-li_raw // P))             # number of 128-row i tiles
    levels = []
    for t in range(ti):
        lo, hi = P * t, P * (t + 1)
        lv = 0
        for a in (1, 2, 3):
            if scls[a] > 0 and istart[a] < hi and istart[a] + scls[a] > lo:
                lv = a
        levels.append(lv)
    return jstart, lj, jext, istart, li_raw, ti, levels


def _groups(tiles):
    """Split a level's tile list into ACT merge groups of 2-3 tiles."""
    out = []
    i = 0
    while i < len(tiles):
        n = 3 if len(tiles) - i == 3 else 2
        out.append(tiles[i : i + n])
        i += n
    return out


def _build(scls):
    """Build + compile the SPMD bass program for given segment sizes."""
    jstart, lj, jext, istart, li_raw, ti, levels = _layout(scls)
    f32 = mybir.dt.float32
    f32r = mybir.dt.float32r

    nc = bacc.Bacc("TRN2", debug=False, enable_asserts=False, num_devices=N_CORES)
    expxj_d = nc.dram_tensor(
        "expxj", [GPC, max(lj, 1)], f32, kind="ExternalInput").ap()
    expnegxi_d = nc.dram_tensor(
        "expnegxi", [P, GPC * max(ti, 1)], f32, kind="ExternalInput").ap()
    onehot_d = nc.dram_tensor(
        "onehot", [P, GPC * max(ti, 1) * NCLS], f32r, kind="ExternalInput").ap()
    # G export: per graph one [4, sum-of-extents] block, one slice per level
    goff = {}
    gtot = 0
    for _a in (1, 2, 3):
        if jext[_a] > 0:
            goff[_a] = gtot
            gtot += jext[_a]
    gtot = max(gtot, 1)
    gout_d = nc.dram_tensor(
        "gout", [GPC, 4, gtot], f32, kind="ExternalOutput").ap()

    with tile.TileContext(nc) as tc:
        with (
            tc.tile_pool(name="sb", bufs=1) as sb,
            tc.tile_pool(name="xrp", bufs=1) as xrp,
            tc.tile_pool(name="tp", bufs=4) as tp,
            tc.tile_pool(name="vp", bufs=3) as vp,
            tc.tile_pool(name="gsp", bufs=2) as gsp,
            tc.tile_pool(name="ps", bufs=2, space="PSUM") as ps,
        ):
            # warm-up: force the Ln act-table load before any DMA-dependent op
            warm = sb.tile([1, 1], f32)
            nc.vector.memset(warm[:], 1.0)
            nc.scalar.activation(warm[:], warm[:], AF.Ln, bias=1.0, scale=1.0)
            # PE p-state warm-up: ~3us of dummy matmuls with no input deps,
            # overlapping the input-DMA head so real matmuls run full speed
            wmm_in = sb.tile([P, CH], mybir.dt.bfloat16)
            wmm_w = sb.tile([P, 4], mybir.dt.bfloat16)
            nc.gpsimd.memset(wmm_in[:], 0.0)
            nc.gpsimd.memset(wmm_w[:], 0.0)
            wmm_ps = ps.tile([4, lj], f32, tag="g", bufs=2, name="wmm_ps")
            for _w in range(8):
                nc.tensor.matmul(wmm_ps[:, :CH], wmm_w[:], wmm_in[:],
                                 start=True, stop=True)

            expnegxi = sb.tile([P, GPC * ti], f32)
            onehot = sb.tile([P, GPC * ti * NCLS], f32r)
            xreps = []
            xjrows = []
            bnds = sorted({jext[a] for a in (1, 2, 3) if jext[a] > 0})
            for g in range(GPC):
                xreps.append(
                    xrp.tile([P, lj], f32, tag=f"xrep{g}", name=f"xrep{g}"))
                xjr = sb.tile([1, lj], f32, tag=f"xjr{g}", name=f"xjr{g}")
                nc.sync.dma_start(xjr[:], expxj_d[g : g + 1, :])
                xjrows.append(xjr)
            nc.sync.dma_start(expnegxi[:], expnegxi_d[:])
            nc.sync.dma_start(onehot[:], onehot_d[:])
            # broadcast the tiny e^{x_j} rows across partitions on the
            # otherwise-idle GPSIMD engine, level-chunked for early start
            half0 = (bnds[0] // 2) if bnds else 0
            for g in range(GPC):
                prev = 0
                chunks = ([half0] + bnds if g == 0 and half0 else
                          (bnds if g == 0 else [lj]))
                for b in chunks:
                    nc.gpsimd.partition_broadcast(
                        xreps[g][:, prev:b], xjrows[g][:, prev:b])
                    prev = b

            # max ACT merge-group width (columns) for t/v buffer sizing
            gw_max = 1
            for a in (1, 2, 3):
                tl = [t for t in range(ti) if levels[t] == a]
                for grp in _groups(tl):
                    gw_max = max(gw_max, len(grp) * jext[a])

            # deferred G export: emit level (g,a)'s copy+DMA after the NEXT
            # level's first group of DVE multiplies, so the copy never sits
            # between ACT and its t-buffer production at a level boundary
            pending = []

            def flush_pending(limit=None):
                n = 0
                while pending and (limit is None or n < limit):
                    fg, fa, fext, fg_ps, fgsb = pending.pop(0)
                    o = goff[fa]
                    nc.vector.tensor_copy(
                        fgsb[:, o : o + fext], fg_ps[:, :fext])
                    nc.sync.dma_start(
                        gout_d[fg, :, o : o + fext], fgsb[:, o : o + fext])
                    n += 1

            for g in range(GPC):
                xrep = xreps[g]
                gsb = gsp.tile([4, gtot], f32, tag="gs", name="gsb")
                order = (1, 2, 3) if g < GPC - 1 else (3, 2, 1)
                for a in order:
                    tiles = [t for t in range(ti) if levels[t] == a]
                    ext = jext[a]
                    if not tiles or ext == 0:
                        continue
                    nch = -(-ext // CH)
                    g_ps = ps.tile([4, lj], f32, tag="g", name="g_ps", bufs=2)
                    fold = (g == 0 and a == 1)
                    done = 0
                    grps = [[t] for t in tiles] if fold else _groups(tiles)
                    for gi, grp in enumerate(grps):
                        gw = len(grp) * ext
                        vbuf = vp.tile([P, gw_max], f32r, tag="v", name="vbuf")
                        if fold:
                            col = g * ti + grp[0]
                            if gi == 0 and half0:
                                nc.scalar.activation(
                                    vbuf[:, :half0], xrep[:, :half0], AF.Ln,
                                    bias=1.0, scale=expnegxi[:, col : col + 1],
                                )
                                nc.scalar.activation(
                                    vbuf[:, half0:ext], xrep[:, half0:ext],
                                    AF.Ln,
                                    bias=1.0, scale=expnegxi[:, col : col + 1],
                                )
                            else:
                                nc.scalar.activation(
                                    vbuf[:, :ext], xrep[:, :ext], AF.Ln,
                                    bias=1.0, scale=expnegxi[:, col : col + 1],
                                )
                        else:
                            tbuf = tp.tile([P, gw_max], f32, tag="t", name="tbuf")
                            for q, t in enumerate(grp):
                                col = g * ti + t
                                nc.vector.tensor_scalar_mul(
                                    tbuf[:, q * ext : (q + 1) * ext],
                                    xrep[:, :ext],
                                    expnegxi[:, col : col + 1],
                                )
                            nc.scalar.activation(
                                vbuf[:, :gw], tbuf[:, :gw], AF.Ln,
                                bias=1.0, scale=1.0,
                            )
                        if gi > 0 or not fold:
                            flush_pending(limit=1)
                        for q, t in enumerate(grp):
                            col = g * ti + t
                            idx = done + q
                            for k in range(nch):
                                k0 = k * CH
                                k1 = min(k0 + CH, ext)
                                nc.tensor.matmul(
                                    g_ps[:, k0:k1],
                                    onehot[:, col * NCLS : (col + 1) * NCLS],
                                    vbuf[:, q * ext + k0 : q * ext + k1],
                                    start=(idx == 0),
                                    stop=(idx == len(tiles) - 1),
                                )
                        done += len(grp)
                    pending.append((g, a, ext, g_ps, gsb))
            flush_pending()
    nc.compile()
    return nc


def _prepare_core(logits, labels, scls):
    """Host-side layout prep for one core's GPC graphs."""
    jstart, lj, jext, istart, li_raw, ti, levels = _layout(scls)
    expxj = np.zeros((GPC, max(lj, 1)), np.float32)
    expnegxi = np.zeros((GPC, P, max(ti, 1)), np.float32)
    onehot = np.zeros((GPC, max(ti, 1), P, NCLS), np.float32)
    for g in range(GPC):
        x = logits[g].astype(np.float64)
        lab = labels[g]
        for c in (0, 1, 2):
            xc = x[lab == c]
            expxj[g, jstart[c] : jstart[c] + xc.size] = np.exp(xc)
        ivals = np.zeros(P * max(ti, 1), np.float64)
        ioh = np.zeros((P * max(ti, 1), NCLS), np.float32)
        for a in (1, 2, 3):
            xa = x[lab == a]
            i0 = istart[a]
            ivals[i0 : i0 + xa.size] = np.exp(-xa)
            ioh[i0 : i0 + xa.size, a] = 1.0
        expnegxi[g] = ivals.reshape(max(ti, 1), P).T.astype(np.float32)
        onehot[g] = ioh.reshape(max(ti, 1), P, NCLS)
    expnegxi_sb = np.ascontiguousarray(
        expnegxi.transpose(1, 0, 2).reshape(P, GPC * max(ti, 1)))
    onehot_sb = np.ascontiguousarray(
        onehot.transpose(2, 0, 1, 3).reshape(P, GPC * max(ti, 1) * NCLS))
    return {"expxj": expxj, "expnegxi": expnegxi_sb, "onehot": onehot_sb}


def _assemble(g_all, counts, scls):
    """Host-side final math from device G matrices. g_all: [B,3,4*mc,CH]."""
    jstart, lj, jext, istart, li_raw, ti, levels = _layout(scls)
    have_level = {a: any(lv == a for lv in levels) and jext[a] > 0
                  for a in (1, 2, 3)}
    # split per-level slices to [B, 3, 4, lj]
    goff = {}
    gtot = 0
    for _a in (1, 2, 3):
        if jext[_a] > 0:
            goff[_a] = gtot
            gtot += jext[_a]
    gm = np.zeros((B, 3, 4, max(lj, 1)), np.float64)
    for a in (1, 2, 3):
        if jext[a] <= 0:
            continue
        o = goff[a]
        gm[:, a - 1, :, : jext[a]] = g_all[:, :, o : o + jext[a]]
    per_graph = np.zeros(B, np.float64)
    for g in range(B):
        n = counts[g]
        means = []
        valids = []
        for lvl in (1, 2, 3):
            s_dev = 0.0
            for a in range(lvl, 4):
                if not have_level.get(a, False):
                    continue
                for c in range(lvl):
                    c0, c1 = jstart[c], jstart[c] + scls[c]
                    if c1 > c0:
                        s_dev += gm[g, a - 1, lvl, c0:c1].sum()
            s_ref = -s_dev
            cnt = float(n[lvl]) * float(n[:lvl].sum())
            valid = cnt > 0
            means.append(s_ref / max(cnt, 1.0) if valid else 0.0)
            valids.append(1.0 if valid else 0.0)
        per_graph[g] = sum(means) / max(sum(valids), 1.0)
    return np.float32(-per_graph.mean())


def kernel(logits, labels):
    logits = np.ascontiguousarray(np.asarray(logits, np.float32))
    labels = np.ascontiguousarray(np.asarray(labels, np.int32))
    assert logits.shape == (B, N) and labels.shape == (B, N)

    counts = np.stack([(labels == c).sum(1) for c in range(NCLS)], axis=1)  # [B,4]
    # even segment sizes keep every extent/chunk slice 4-byte aligned in bf16
    scls = tuple(int(counts[:, c].max() + 1) // 2 * 2 for c in range(NCLS))

    jstart, lj, jext, istart, li_raw, ti, levels = _layout(scls)
    if ti == 0 or lj == 0:
        # no (pos, neg) pairs exist anywhere: every level invalid -> loss 0
        return np.float32(-0.0)

    if scls not in _BUILD_CACHE:
        _BUILD_CACHE[scls] = _build(scls)
    nc = _BUILD_CACHE[scls]

    in_maps = [
        _prepare_core(logits[c * GPC : (c + 1) * GPC],
                      labels[c * GPC : (c + 1) * GPC], scls)
        for c in range(N_CORES)
    ]
    res = run_bass_kernel_spmd(nc, in_maps, list(range(N_CORES)))
    g_all = np.concatenate(
        [res.results[c]["gout"] for c in range(N_CORES)], axis=0)
    return _assemble(g_all, counts, scls)


if __name__ == "__main__":
    rng = np.random.default_rng(0)
    lg = rng.normal(size=(B, N)).astype(np.float32)
    lb = rng.integers(0, NCLS, size=(B, N)).astype(np.int32)
    print(kernel(lg, lb))


# revision 37
# speedup vs baseline: 1.0051x; 1.0051x over previous
"""Trainium2 Bass kernel for the BPR-style soft-label pairwise loss.

Reference math (per graph g of B=16, N=2048 nodes, labels in {0..3}):
  for lvl in 1..3:
    s_lvl   = sum_{i: lab=lvl} sum_{j: lab<lvl} log_sigmoid(x_i - x_j)
    cnt_lvl = n_lvl * n_{<lvl};  mean_lvl = s_lvl/cnt_lvl if cnt>0 else 0
  per_graph = sum(mean_lvl) / max(#valid, 1);  loss = -mean_g(per_graph)

Kernel strategy (data-parallel, 2 graphs per core on 8 cores):
  -log_sigmoid(x_i - x_j) = ln(1 + e^{x_j} * e^{-x_i})
  The host sorts each graph's nodes by label into a class-segmented layout
  that is uniform across graphs (segment size = max class count over all
  graphs rounded to even; padded slots carry e^{x}=0 so ln(1+0)=0 contributes
  nothing).  Only pairs with lab_i > lab_j are ever evaluated: i-tiles cover
  classes 1..3, each with j-extent = end of class (lab_i - 1)'s segment, so
  the device does ~3/8 of the dense N^2 transcendental work.

  Per 128-row i-tile the DVE forms t = xrep * e^{-x_i} (bf16 tensor_scalar,
  4x mode) into a grouped buffer; one ScalarE Ln instruction (bias=1) covers
  a whole group of tiles (ACT cost is per-column, so fewer instructions means
  less fixed overhead).  A one-hot [128,4] bf16 matmul contracts the i
  dimension class-resolved into PSUM, with each 512-wide j-chunk accumulating
  into its own 4-row partition band of a single PSUM bank; one DVE copy
  drains a level's whole G to SBUF, a DMA exports it, and the O(N) segment
  sums plus O(1) count/divide/average logic run on host in float64.
"""

import os
import sys

import numpy as np

for _p in ("/root/.axon_site/_ro/trn_rl_repo", "/opt/trn_rl_repo"):
    if os.path.isdir(_p) and _p not in sys.path:
        sys.path.append(_p)

import concourse.bacc as bacc
import concourse.mybir as mybir
import concourse.tile as tile
from concourse.bass_utils import run_bass_kernel_spmd

B, N, NCLS = 16, 2048, 4
N_CORES = 8
GPC = B // N_CORES  # graphs per core
P = 128
CH = 512           # PSUM bank chunk (f32 columns)
AF = mybir.ActivationFunctionType

_BUILD_CACHE = {}


def _layout(scls):
    """Derive the uniform class-segmented layout from per-class segment sizes."""
    s0, s1, s2, s3 = scls
    jstart = [0, s0, s0 + s1, s0 + s1 + s2]  # segment starts for j classes 0..2
    lj = s0 + s1 + s2                        # j layout length (classes 0..2)
    jext = {1: jstart[1], 2: jstart[2], 3: lj}  # j extent per i level
    istart = {1: 0, 2: s1, 3: s1 + s2}       # i layout: classes 1..3
    li_raw = s1 + s2 + s3
    ti = max(0, -(-li_raw // P))             # number of 128-row i tiles
    levels = []
    for t in range(ti):
        lo, hi = P * t, P * (t + 1)
        lv = 0
        for a in (1, 2, 3):
            if scls[a] > 0 and istart[a] < hi and istart[a] + scls[a] > lo:
                lv = a
        levels.append(lv)
    return jstart, lj, jext, istart, li_raw, ti, levels


def _groups(tiles):
    """Split a level's tile list into ACT merge groups of 2-3 tiles."""
    out = []
    i = 0
    while i < len(tiles):
        n = 3 if len(tiles) - i == 3 else 2
        out.append(tiles[i : i + n])
        i += n
    return out


def _build(scls):
    """Build + compile the SPMD bass program for given segment sizes."""
    jstart, lj, jext, istart, li_raw, ti, levels = _layout(scls)
    f32 = mybir.dt.float32
    f32r = mybir.dt.float32r

    nc = bacc.Bacc("TRN2", debug=False, enable_asserts=False, num_devices=N_CORES)
    expxj_d = nc.dram_tensor(
        "expxj", [GPC, max(lj, 1)], f32, kind="ExternalInput").ap()
    expnegxi_d = nc.dram_tensor(
        "expnegxi", [P, GPC * max(ti, 1)], f32, kind="ExternalInput").ap()
    onehot_d = nc.dram_tensor(
        "onehot", [P, GPC * max(ti, 1) * NCLS], f32r, kind="ExternalInput").ap()
    # G export: per graph one [4, sum-of-extents] block, one slice per level
    goff = {}
    gtot = 0
    for _a in (1, 2, 3):
        if jext[_a] > 0:
            goff[_a] = gtot
            gtot += jext[_a]
    gtot = max(gtot, 1)
    gout_d = nc.dram_tensor(
        "gout", [GPC, 4, gtot], f32, kind="ExternalOutput").ap()

    with tile.TileContext(nc) as tc:
        with (
            tc.tile_pool(name="sb", bufs=1) as sb,
            tc.tile_pool(name="xrp", bufs=1) as xrp,
            tc.tile_pool(name="tp", bufs=4) as tp,
            tc.tile_pool(name="vp", bufs=3) as vp,
            tc.tile_pool(name="gsp", bufs=2) as gsp,
            tc.tile_pool(name="ps", bufs=2, space="PSUM") as ps,
        ):
            # warm-up: force the Ln act-table load before any DMA-dependent op
            warm = sb.tile([1, 1], f32)
            nc.vector.memset(warm[:], 1.0)
            nc.scalar.activation(warm[:], warm[:], AF.Ln, bias=1.0, scale=1.0)
            # PE p-state warm-up: ~3us of dummy matmuls with no input deps,
            # overlapping the input-DMA head so real matmuls run full speed
            wmm_in = sb.tile([P, CH], mybir.dt.bfloat16)
            wmm_w = sb.tile([P, 4], mybir.dt.bfloat16)
            nc.gpsimd.memset(wmm_in[:], 0.0)
            nc.gpsimd.memset(wmm_w[:], 0.0)
            wmm_ps = ps.tile([4, lj], f32, tag="g", bufs=2, name="wmm_ps")
            for _w in range(8):
                nc.tensor.matmul(wmm_ps[:, :CH], wmm_w[:], wmm_in[:],
                                 start=True, stop=True)

            expnegxi = sb.tile([P, GPC * ti], f32)
            onehot = sb.tile([P, GPC * ti * NCLS], f32r)
            xreps = []
            xjrows = []
            bnds = sorted({jext[a] for a in (1, 2, 3) if jext[a] > 0})
            for g in range(GPC):
                xreps.append(
                    xrp.tile([P, lj], f32, tag=f"xrep{g}", name=f"xrep{g}"))
                xjr = sb.tile([1, lj], f32, tag=f"xjr{g}", name=f"xjr{g}")
                nc.sync.dma_start(xjr[:], expxj_d[g : g + 1, :])
                xjrows.append(xjr)
            nc.sync.dma_start(expnegxi[:], expnegxi_d[:])
            nc.sync.dma_start(onehot[:], onehot_d[:])
            # broadcast the tiny e^{x_j} rows across partitions on the
            # otherwise-idle GPSIMD engine, level-chunked for early start
            for g in range(GPC):
                prev = 0
                for b in bnds if g == 0 else [lj]:
                    nc.gpsimd.partition_broadcast(
                        xreps[g][:, prev:b], xjrows[g][:, prev:b])
                    prev = b

            # max ACT merge-group width (columns) for t/v buffer sizing
            gw_max = 1
            for a in (1, 2, 3):
                tl = [t for t in range(ti) if levels[t] == a]
                for grp in _groups(tl):
                    gw_max = max(gw_max, len(grp) * jext[a])

            # deferred G export: emit level (g,a)'s copy+DMA after the NEXT
            # level's first group of DVE multiplies, so the copy never sits
            # between ACT and its t-buffer production at a level boundary
            pending = []

            def flush_pending(limit=None):
                n = 0
                while pending and (limit is None or n < limit):
                    fg, fa, fext, fg_ps, fgsb = pending.pop(0)
                    o = goff[fa]
                    nc.vector.tensor_copy(
                        fgsb[:, o : o + fext], fg_ps[:, :fext])
                    nc.sync.dma_start(
                        gout_d[fg, :, o : o + fext], fgsb[:, o : o + fext])
                    n += 1

            for g in range(GPC):
                xrep = xreps[g]
                gsb = gsp.tile([4, gtot], f32, tag="gs", name="gsb")
                order = (1, 2, 3) if g < GPC - 1 else (3, 2, 1)
                for a in order:
                    tiles = [t for t in range(ti) if levels[t] == a]
                    ext = jext[a]
                    if not tiles or ext == 0:
                        continue
                    nch = -(-ext // CH)
                    g_ps = ps.tile([4, lj], f32, tag="g", name="g_ps", bufs=2)
                    fold = (g == 0 and a == 1)
                    done = 0
                    grps = [[t] for t in tiles] if fold else _groups(tiles)
                    for gi, grp in enumerate(grps):
                        gw = len(grp) * ext
                        vbuf = vp.tile([P, gw_max], f32r, tag="v", name="vbuf")
                        if fold:
                            col = g * ti + grp[0]
                            nc.scalar.activation(
                                vbuf[:, :ext], xrep[:, :ext], AF.Ln,
                                bias=1.0, scale=expnegxi[:, col : col + 1],
                            )
                        else:
                            tbuf = tp.tile([P, gw_max], f32, tag="t", name="tbuf")
                            for q, t in enumerate(grp):
                                col = g * ti + t
                                nc.vector.tensor_scalar_mul(
                                    tbuf[:, q * ext : (q + 1) * ext],
                                    xrep[:, :ext],
                                    expnegxi[:, col : col + 1],
                                )
                            nc.scalar.activation(
                                vbuf[:, :gw], tbuf[:, :gw], AF.Ln,
                                bias=1.0, scale=1.0,
                            )
                        if gi > 0 or not fold:
                            flush_pending(limit=1)
                        for q, t in enumerate(grp):
                            col = g * ti + t
                            idx = done + q
                            for k in range(nch):
                                k0 = k * CH
                                k1 = min(k0 + CH, ext)
                                nc.tensor.matmul(
                                    g_ps[:, k0:k1],
                                    onehot[:, col * NCLS : (col + 1) * NCLS],
                                    vbuf[:, q * ext + k0 : q * ext + k1],
                                    start=(idx == 0),
                                    stop=(idx == len(tiles) - 1),
                                )
                        done += len(grp)
                    pending.append((g, a, ext, g_ps, gsb))
            flush_pending()
    nc.compile()
    return nc


def _prepare_core(logits, labels, scls):
    """Host-side layout prep for one core's GPC graphs."""
    jstart, lj, jext, istart, li_raw, ti, levels = _layout(scls)
    expxj = np.zeros((GPC, max(lj, 1)), np.float32)
    expnegxi = np.zeros((GPC, P, max(ti, 1)), np.float32)
    onehot = np.zeros((GPC, max(ti, 1), P, NCLS), np.float32)
    for g in range(GPC):
        x = logits[g].astype(np.float64)
        lab = labels[g]
        for c in (0, 1, 2):
            xc = x[lab == c]
            expxj[g, jstart[c] : jstart[c] + xc.size] = np.exp(xc)
        ivals = np.zeros(P * max(ti, 1), np.float64)
        ioh = np.zeros((P * max(ti, 1), NCLS), np.float32)
        for a in (1, 2, 3):
            xa = x[lab == a]
            i0 = istart[a]
            ivals[i0 : i0 + xa.size] = np.exp(-xa)
            ioh[i0 : i0 + xa.size, a] = 1.0
        expnegxi[g] = ivals.reshape(max(ti, 1), P).T.astype(np.float32)
        onehot[g] = ioh.reshape(max(ti, 1), P, NCLS)
    expnegxi_sb = np.ascontiguousarray(
        expnegxi.transpose(1, 0, 2).reshape(P, GPC * max(ti, 1)))
    onehot_sb = np.ascontiguousarray(
        onehot.transpose(2, 0, 1, 3).reshape(P, GPC * max(ti, 1) * NCLS))
    return {"expxj": expxj, "expnegxi": expnegxi_sb, "onehot": onehot_sb}


def _assemble(g_all, counts, scls):
    """Host-side final math from device G matrices. g_all: [B,3,4*mc,CH]."""
    jstart, lj, jext, istart, li_raw, ti, levels = _layout(scls)
    have_level = {a: any(lv == a for lv in levels) and jext[a] > 0
                  for a in (1, 2, 3)}
    # split per-level slices to [B, 3, 4, lj]
    goff = {}
    gtot = 0
    for _a in (1, 2, 3):
        if jext[_a] > 0:
            goff[_a] = gtot
            gtot += jext[_a]
    gm = np.zeros((B, 3, 4, max(lj, 1)), np.float64)
    for a in (1, 2, 3):
        if jext[a] <= 0:
            continue
        o = goff[a]
        gm[:, a - 1, :, : jext[a]] = g_all[:, :, o : o + jext[a]]
    per_graph = np.zeros(B, np.float64)
    for g in range(B):
        n = counts[g]
        means = []
        valids = []
        for lvl in (1, 2, 3):
            s_dev = 0.0
            for a in range(lvl, 4):
                if not have_level.get(a, False):
                    continue
                for c in range(lvl):
                    c0, c1 = jstart[c], jstart[c] + scls[c]
                    if c1 > c0:
                        s_dev += gm[g, a - 1, lvl, c0:c1].sum()
            s_ref = -s_dev
            cnt = float(n[lvl]) * float(n[:lvl].sum())
            valid = cnt > 0
            means.append(s_ref / max(cnt, 1.0) if valid else 0.0)
            valids.append(1.0 if valid else 0.0)
        per_graph[g] = sum(means) / max(sum(valids), 1.0)
    return np.float32(-per_graph.mean())


def kernel(logits, labels):
    logits = np.ascontiguousarray(np.asarray(logits, np.float32))
    labels = np.ascontiguousarray(np.asarray(labels, np.int32))
    assert logits.shape == (B, N) and labels.shape == (B, N)

    counts = np.stack([(labels == c).sum(1) for c in range(NCLS)], axis=1)  # [B,4]
    # even segment sizes keep every extent/chunk slice 4-byte aligned in bf16
    scls = tuple(int(counts[:, c].max() + 1) // 2 * 2 for c in range(NCLS))

    jstart, lj, jext, istart, li_raw, ti, levels = _layout(scls)
    if ti == 0 or lj == 0:
        # no (pos, neg) pairs exist anywhere: every level invalid -> loss 0
        return np.float32(-0.0)

    if scls not in _BUILD_CACHE:
        _BUILD_CACHE[scls] = _build(scls)
    nc = _BUILD_CACHE[scls]

    in_maps = [
        _prepare_core(logits[c * GPC : (c + 1) * GPC],
                      labels[c * GPC : (c + 1) * GPC], scls)
        for c in range(N_CORES)
    ]
    res = run_bass_kernel_spmd(nc, in_maps, list(range(N_CORES)))
    g_all = np.concatenate(
        [res.results[c]["gout"] for c in range(N_CORES)], axis=0)
    return _assemble(g_all, counts, scls)


if __name__ == "__main__":
    rng = np.random.default_rng(0)
    lg = rng.normal(size=(B, N)).astype(np.float32)
    lb = rng.integers(0, NCLS, size=(B, N)).astype(np.int32)
    print(kernel(lg, lb))
